# revision 46
# baseline (speedup 1.0000x reference)
"""Causal multi-head self-attention with RoPE on 8 NeuronCores.

Sharding (hardcoded): core c -> batch b = c // 2, head-group hg = c % 2.
Each core:
  - projects its batch's x with column-sharded WQ/WK/WV (8 heads = 512 dims),
  - applies RoPE (host-precomputed bf16 cos/sin tables; adjacent-pair
    swap via stream_shuffle),
  - runs causal attention for its 8 heads in transposed layout
    (S^T = [k, q]; softmax denominator comes free from a ones-column
    appended to V; normalization is broadcast via a DRAM bounce),
  - applies the row-sharded WO projection -> bf16 partial [T, D] output.
Host sums the two partials per batch (the "all-reduce after WO").

The schedule is a fine-grained software pipeline built around the
attention i-loop, which is paced by the ACT exp stream (~1.04us/i vs
~0.85us/i of S+PV matmuls on PE). All projection / V / WO matmuls are
split into single-matmul micro-ops on a FIFO work queue tagged with
the attention phase they must precede; each i-iteration of every
attention chunk pops up to pops_per_i PE micro-ops off the queue,
keeping PE busy through the exp-paced stretches. Only WO ops carry a
lookahead cap (req <= phase+1: they must never be emitted before the
norm that writes their ot block); hazard-free proj/V ops pop
arbitrarily far ahead, so their RoPE/evict chains finish well before
their attention consumers. PV matmuls lag `lag` i-iterations behind the
S matmuls so the exp/trim -> PV dependency never head-of-line blocks
the in-order PE queue. PSUM: the S ring (2x2 banks) and the o_a/o_b
accumulators (2x1) are separate pools from the filler chains (2x1),
so a popped filler allocation never waits on an o tile held through
an i-loop. Input DMA is consolidated into ~14 multi-dim copies
(HWDGE descriptor-generation is a fixed cost per copy and paces the
prologue); the final chunk's softmax normalization broadcasts on the
then-idle PE (ones[1,64]^T @ recip) instead of the DRAM bounce so
the tail WO tiles are not stalled behind it.

All inputs are packed into ONE DRAM tensor per core and the output is
not passed as a zero-placeholder operand (the kernel writes every
element, so the runtime-allocated uninit result buffer is fine): the
per-call execution overhead through the axon PJRT path scales with the
number of I/O buffers (~65us each) plus ~15us/MB, so both buffer count
and bytes are minimized. Packed layout (bf16, [2176, 2048]):
  rows    0:1024  xT (x[b].T, [d, t])
  rows 1024:2048  wqT | wkT | wvT | woT-halves (cols 0:512, 512:1024,
                  1024:1536, 1536:2048)
  rows 2048:2176  cos | sin ([64, 2048] each; table partition rows repeat
                  with period 64, so each is DMA'd to both halves)
The triangular trim mask is generated on-chip (memset + affine_select).
"""

import numpy as np
import ml_dtypes

B, T, D, H = 4, 2048, 1024, 16
DK = 64
HLOC = 8          # heads per core
E = HLOC * DK     # 512, local projection width
NCORES = 8
THETA = 10000.0

_BF16 = ml_dtypes.bfloat16

# packed-input row offsets. WO's two column-halves sit in the otherwise
# unused cols 1536:2048 of the W row-block; the triangular trim mask is
# generated on-chip with affine_select, so the buffer stays tight.
_ROW_X = 0
_ROW_W = 1024
_ROW_TBL = 2048
_PK_ROWS = 2176
_PK_COLS = 2048

_cache = {}


def _build(
    t=T,
    hloc=HLOC,
    d=D,
    reps=1,
    pops_per_i=2,
    pops_wo=None,
    pops_last=2,
    wo_delay=4,
    lag=4,
    dma_halves=False,
    evict_split=True,
    pool_mul=False,
    rope_add_pool=False,
    trim_dve=True,
    dma_fine=False,
    dma_hybrid=False,
    vfin_pool=False,
    warmup_mm=0,
):
    from collections import deque
    from contextlib import ExitStack

    import concourse.bacc as bacc
    import concourse.bass as bass  # noqa: F401
    import concourse.mybir as mybir
    import concourse.tile as tile

    f32 = mybir.dt.float32
    bf16 = mybir.dt.bfloat16
    Exp = mybir.ActivationFunctionType.Exp
    Copy = mybir.ActivationFunctionType.Copy

    e = hloc * DK
    npair = hloc // 2       # head-pair tiles in QT/KT/OT
    dsub = d // 128         # contraction subtiles for projections
    tq = t // 512           # 512-wide q chunks
    tk = t // 128           # 128-wide k tiles
    swap_mask = [i ^ 1 for i in range(32)]
    e_v = min(512, e)

    nc = bacc.Bacc(
        "TRN2", target_bir_lowering=False, debug=False, enable_partition_id=False
    )

    pk = nc.declare_dram_parameter("pk", [_PK_ROWS, _PK_COLS], bf16, False).ap()
    y = nc.declare_dram_parameter("y", [t, d], bf16, True).ap()
    scr = nc.dram_tensor("scr", [hloc, t], bf16)  # denom-recip bounce for bcast

    def pk_view(row, col, ap_dims):
        """Multi-dim view into the packed DRAM input: one DMA per logical
        tensor instead of one per 128-row slab (HWDGE cost is per copy)."""
        return bass.AP(
            tensor=pk.tensor, offset=row * _PK_COLS + col, ap=list(ap_dims)
        )

    with tile.TileContext(nc) as tc:
        with ExitStack() as ctx:
            const = ctx.enter_context(tc.tile_pool(name="const", bufs=1))
            ptpool = ctx.enter_context(tc.tile_pool(name="ptp", bufs=8))
            normp = ctx.enter_context(tc.tile_pool(name="normp", bufs=4))
            ysbp = ctx.enter_context(tc.tile_pool(name="ysbp", bufs=3))

            wq_sb = const.tile([128, dsub, e], bf16)
            wk_sb = const.tile([128, dsub, e], bf16)
            wv_sb = const.tile([128, dsub, e], bf16)
            wo_sb = const.tile([128, e // 128, d], bf16)
            trim_sb = const.tile([128, 128], bf16)
            ones_sb = const.tile([1, 64], bf16)
            qt_sb = const.tile([128, npair, t], bf16)
            kt_sb = const.tile([128, npair, t], bf16)
            v_sb = const.tile([128, tk, hloc, DK + 1], bf16)
            ot_sb = const.tile([128, npair, t], bf16)

            # DMA order follows first use in the pipelined schedule:
            # wq -> xt chunk0 -> wk -> wv -> rope tables -> xt rest -> trim
            # -> wo. Q/K/V projections and pair-0 attention start while the
            # later loads are still in flight.
            xt_sb = const.tile([128, dsub, t], bf16)
            cos_sb = const.tile([128, t], bf16)
            sin_sb = const.tile([128, t], bf16)
            tch = 512
            # trim[r, c] = 1.0 if c >= r else 0.0 (upper triangle), built
            # on-chip
            nc.vector.memset(trim_sb, 1.0)
            nc.vector.memset(ones_sb, 1.0)
            nc.gpsimd.affine_select(
                trim_sb,
                trim_sb,
                pattern=[[1, 128]],
                compare_op=mybir.AluOpType.is_ge,
                fill=0.0,
                base=0,
                channel_multiplier=-1,
            )
            # whole-tensor DMAs via multi-dim views: HWDGE descriptor
            # generation is a fixed ~0.6us per COPY, and the issue stream
            # paces the whole prologue, so copy count is minimized (14
            # input copies). Order follows first use: wq+x chunk0 (Q0),
            # tables (Q0's RoPE), wk (K0), wv (V tiles), x rest, wo.
            row = [_PK_COLS, 128]
            dsl = [128 * _PK_COLS, dsub]
            if dma_fine or dma_hybrid:
                # per-slab copies for the prologue gate: the first Q0
                # matmul only needs wq[ds0]+x[ds0] (256KB), not the full
                # 2MB consolidated pair
                for ds_ in range(dsub):
                    r = _ROW_W + ds_ * 128
                    nc.sync.dma_start(wq_sb[:, ds_, :], pk[r : r + 128, 0:e])
                    nc.sync.dma_start(
                        xt_sb[:, ds_, 0:tch],
                        pk[ds_ * 128 : (ds_ + 1) * 128, 0:tch],
                    )
            elif dma_halves:
                hd = dsub // 2
                dsl_h = [128 * _PK_COLS, hd]
                # wq/x chunk0 in interleaved halves so the first projection
                # accumulation starts after ~1MB instead of 2MB
                nc.sync.dma_start(
                    wq_sb[:, 0:hd, :], pk_view(_ROW_W, 0, [row, dsl_h, [1, e]])
                )
                nc.sync.dma_start(
                    xt_sb[:, 0:hd, 0:tch], pk_view(0, 0, [row, dsl_h, [1, tch]])
                )
                nc.sync.dma_start(
                    wq_sb[:, hd:dsub, :],
                    pk_view(_ROW_W + hd * 128, 0, [row, dsl_h, [1, e]]),
                )
                nc.sync.dma_start(
                    xt_sb[:, hd:dsub, 0:tch],
                    pk_view(hd * 128, 0, [row, dsl_h, [1, tch]]),
                )
            else:
                nc.sync.dma_start(
                    wq_sb[:, :, :], pk_view(_ROW_W, 0, [row, dsl, [1, e]])
                )
                nc.sync.dma_start(
                    xt_sb[:, :, 0:tch], pk_view(0, 0, [row, dsl, [1, tch]])
                )
            # tables next: Q0's RoPE finisher (DVE) needs cos/sin as soon
            # as the first projection chunk's matmuls finish
            nc.sync.dma_start(cos_sb[0:64, :], pk[_ROW_TBL : _ROW_TBL + 64, :])
            nc.sync.dma_start(cos_sb[64:128, :], pk[_ROW_TBL : _ROW_TBL + 64, :])
            nc.sync.dma_start(
                sin_sb[0:64, :], pk[_ROW_TBL + 64 : _ROW_TBL + 128, :]
            )
            nc.sync.dma_start(
                sin_sb[64:128, :], pk[_ROW_TBL + 64 : _ROW_TBL + 128, :]
            )
            if dma_fine:
                for ds_ in range(dsub):
                    r = _ROW_W + ds_ * 128
                    nc.sync.dma_start(
                        wk_sb[:, ds_, :], pk[r : r + 128, e : 2 * e]
                    )
                for ds_ in range(dsub):
                    r = _ROW_W + ds_ * 128
                    nc.sync.dma_start(
                        wv_sb[:, ds_, :], pk[r : r + 128, 2 * e : 3 * e]
                    )
                nc.vector.memset(v_sb[:, :, :, DK : DK + 1], 1.0)
                for t0 in range(tch, t, tch):
                    for ds_ in range(dsub):
                        nc.sync.dma_start(
                            xt_sb[:, ds_, t0 : t0 + tch],
                            pk[ds_ * 128 : (ds_ + 1) * 128, t0 : t0 + tch],
                        )
                for dp in range(e // 128):
                    for dh in range(2):
                        r = _ROW_W + dh * 512 + dp * 128
                        nc.sync.dma_start(
                            wo_sb[:, dp, dh * 512 : (dh + 1) * 512],
                            pk[r : r + 128, 3 * e : 4 * e],
                        )
            else:
                nc.sync.dma_start(
                    wk_sb[:, :, :], pk_view(_ROW_W, e, [row, dsl, [1, e]])
                )
                nc.sync.dma_start(
                    wv_sb[:, :, :], pk_view(_ROW_W, 2 * e, [row, dsl, [1, e]])
                )
                nc.vector.memset(v_sb[:, :, :, DK : DK + 1], 1.0)
                nc.sync.dma_start(
                    xt_sb[:, :, tch : 2 * tch],
                    pk_view(0, tch, [row, dsl, [1, tch]]),
                )
                nc.sync.dma_start(
                    xt_sb[:, :, 2 * tch : t],
                    pk_view(0, 2 * tch, [row, dsl, [1, t - 2 * tch]]),
                )
                for dh in range(2):
                    nc.sync.dma_start(
                        wo_sb[:, :, dh * 512 : (dh + 1) * 512],
                        pk_view(
                            _ROW_W + dh * 512,
                            3 * e,
                            [row, [128 * _PK_COLS, e // 128], [1, 512]],
                        ),
                    )

            for _rep in range(reps):
                with ExitStack() as c1:
                    rope = c1.enter_context(tc.tile_pool(name="rope", bufs=3))
                    # PSUM budget (8 banks): "s" 2x[128,1024] = 4 banks for
                    # attention S^T psum; "o" 2x[128,512] = 2 banks for the
                    # attention o accumulators; "f" 2x[128,512] = 2 banks
                    # for the filler chains (projection / V / WO psum).
                    # Separate o and filler rings keep a popped filler's
                    # allocation from ever waiting on an o-tile held through
                    # a whole i-loop; the FIFO keeps chains contiguous so at
                    # most 2 filler tiles are ever in flight.
                    spsum = c1.enter_context(
                        tc.tile_pool(name="spsum", bufs=2, space="PSUM")
                    )
                    opsum = c1.enter_context(
                        tc.tile_pool(name="opsum", bufs=2, space="PSUM")
                    )
                    fpsum = c1.enter_context(
                        tc.tile_pool(name="fpsum", bufs=2, space="PSUM")
                    )

                    ph = min(512, t)           # projection t-chunk (1 PSUM bank)
                    nh = t // ph               # chunks per e-tile

                    if _rep == 0 and warmup_mm:
                        # p-state warm-up: PE idles ~6us at the start
                        # waiting for the first wq/x DMAs, and the DVFS
                        # ramp then slows the first real matmuls (full
                        # clock only after ~3us of continuous busy). Run
                        # dummy matmuls on a zeroed tile through the
                        # otherwise-dead ramp window so the real stream
                        # starts at speed.
                        wz = rope.tile([128, 512], bf16, tag="sw2", name="wz")
                        nc.vector.memset(wz, 0.0)
                        wu_ps = fpsum.tile([128, 512], f32, tag="f", name="wu_ps")
                        for wi in range(warmup_mm):
                            nc.tensor.matmul(
                                wu_ps[:, :],
                                lhsT=wz[:, 0:128],
                                rhs=wz[:, :],
                                start=(wi == 0),
                                stop=(wi == warmup_mm - 1),
                            )
                        nc.vector.tensor_copy(wz, wu_ps)

                    # ---- micro-op generators -------------------------------
                    # Each returns a list of (is_pe, closure). Ops of one
                    # chain stay contiguous in the FIFO, so at most one
                    # extra PSUM po tile is in flight beyond o_a/o_b.

                    def proj_ops(pr, which, ch):
                        """Q or K projection chunk + RoPE, split into dsub
                        single matmuls plus one finisher op (DVE shuffle +
                        cos-mul, Pool sin-mul, DVE add)."""
                        wsb, dst = (
                            (wq_sb, qt_sb) if which == 0 else (wk_sb, kt_sb)
                        )
                        c0 = ch * ph
                        st = {}

                        def mk_mm(ds_):
                            def f():
                                if ds_ == 0:
                                    st["ps"] = fpsum.tile([128, ph], f32, tag="f", name="q_ps")
                                nc.tensor.matmul(
                                    st["ps"][:, :],
                                    lhsT=wsb[:, ds_, pr * 128 : (pr + 1) * 128],
                                    rhs=xt_sb[:, ds_, c0 : c0 + ph],
                                    start=(ds_ == 0),
                                    stop=(ds_ == dsub - 1),
                                )

                            return f

                        def fin():
                            q_ps = st["ps"]
                            sw = rope.tile([128, ph], f32, tag="sw")
                            nc.vector.stream_shuffle(sw, q_ps, mask=swap_mask)
                            nc.vector.tensor_mul(
                                dst[:, pr, c0 : c0 + ph],
                                q_ps,
                                cos_sb[:, c0 : c0 + ph],
                            )
                            # Pool does the sin product AND the final add:
                            # DVE per chunk (shuffle + cos-mul ~1.6us) then
                            # stays under PE's 1.7us/chunk, so filler bursts
                            # never stall the PE on the fpsum ring.
                            sw2 = rope.tile([128, ph], bf16, tag="sw2")
                            nc.gpsimd.tensor_mul(
                                sw2, sw, sin_sb[:, c0 : c0 + ph]
                            )
                            add_eng = nc.gpsimd if rope_add_pool else nc.vector
                            add_eng.tensor_add(
                                dst[:, pr, c0 : c0 + ph],
                                dst[:, pr, c0 : c0 + ph],
                                sw2,
                            )

                        return [(True, mk_mm(ds_)) for ds_ in range(dsub)] + [
                            (False, fin)
                        ]

                    def v_ops(it):
                        """V projection tile: dsub matmuls into a po tile +
                        ACT psum eviction (Copy shares the exp table, so no
                        act-table reloads)."""
                        st = {}

                        def mk_mm(ds_):
                            def f():
                                if ds_ == 0:
                                    st["ps"] = fpsum.tile(
                                        [128, e_v], f32, tag="f", name="v_ps"
                                    )
                                nc.tensor.matmul(
                                    st["ps"][:, :],
                                    lhsT=xt_sb[:, ds_, it * 128 : (it + 1) * 128],
                                    rhs=wv_sb[:, ds_, :e_v],
                                    start=(ds_ == 0),
                                    stop=(ds_ == dsub - 1),
                                )

                            return f

                        def fin():
                            if vfin_pool:
                                # keep the eviction off the ACT exp stream
                                # (pair-0 ACT runs at ~the PE pace already)
                                nc.gpsimd.tensor_copy(
                                    v_sb[:, it, :, 0:DK],
                                    st["ps"][:, :].rearrange(
                                        "p (h k) -> p h k", h=hloc
                                    ),
                                )
                            else:
                                nc.scalar.activation(
                                    v_sb[:, it, :, 0:DK],
                                    st["ps"][:, :].rearrange(
                                        "p (h k) -> p h k", h=hloc
                                    ),
                                    Copy,
                                )

                        return [(True, mk_mm(ds_)) for ds_ in range(dsub)] + [
                            (False, fin)
                        ]

                    def wo_ops(it, evict_act=False):
                        """WO tile for output rows it*128..: two 512-wide
                        column halves, each e//128 accumulating matmuls plus
                        an eviction, then the output DMA."""
                        st = {}
                        ops = []

                        def mk_mm(ec, dp):
                            def f():
                                if ec == 0 and dp == 0:
                                    st["ysb"] = ysbp.tile([128, d], bf16, tag="ysb", name="y_sb")
                                if dp == 0:
                                    st["yps"] = fpsum.tile(
                                        [128, 512], f32, tag="f", name="y_ps"
                                    )
                                nc.tensor.matmul(
                                    st["yps"][:, :],
                                    lhsT=ot_sb[:, dp, it * 128 : (it + 1) * 128],
                                    rhs=wo_sb[:, dp, ec * 512 : (ec + 1) * 512],
                                    start=(dp == 0),
                                    stop=(dp == e // 128 - 1),
                                )

                            return f

                        def mk_fin(ec):
                            def f():
                                if evict_act or (evict_split and ec == 0):
                                    # ec0 on ACT (slack beside the exp
                                    # stream), ec1 on DVE: neither engine
                                    # becomes the pair-3 pacer
                                    nc.scalar.activation(
                                        st["ysb"][:, ec * 512 : (ec + 1) * 512],
                                        st["yps"][:, :],
                                        Copy,
                                    )
                                else:
                                    nc.vector.tensor_copy(
                                        st["ysb"][:, ec * 512 : (ec + 1) * 512],
                                        st["yps"][:, :],
                                    )
                                # per-half DMA starts the output drain while
                                # the second half still computes
                                nc.sync.dma_start(
                                    y[
                                        it * 128 : (it + 1) * 128,
                                        ec * 512 : (ec + 1) * 512,
                                    ],
                                    st["ysb"][:, ec * 512 : (ec + 1) * 512],
                                )

                            return f

                        for ec in range(d // 512):
                            for dp in range(e // 128):
                                ops.append((True, mk_mm(ec, dp)))
                            ops.append((False, mk_fin(ec)))
                        return ops

                    # ---- work queue ---------------------------------------
                    # req = phase index (pr*tq + j) the ops must precede.
                    # Queue order is by req, so FIFO pops respect both PSUM
                    # ring discipline and write-before-read emission order.
                    work = deque()
                    nphase = npair * tq

                    def phase_idx(pr, j):
                        return pr * tq + j

                    for j in range(tq):
                        r = phase_idx(0, j)
                        for op in proj_ops(0, 0, j):
                            work.append((r, False, op))
                        for op in proj_ops(0, 1, j):
                            work.append((r, False, op))
                        for it in range(4 * j, 4 * j + 4):
                            for op in v_ops(it):
                                work.append((r, False, op))
                    for pr in range(1, npair):
                        for ch in range(nh):
                            r = phase_idx(pr, ch)
                            for op in proj_ops(pr, 0, ch):
                                work.append((r, False, op))
                            for op in proj_ops(pr, 1, ch):
                                work.append((r, False, op))
                    # WO for q-chunk j-1 pops during phase (3, j); the
                    # first wo_delay i-iterations don't pop, giving the
                    # norm DRAM-bounce of chunk j-1 time to land.
                    for j in range(1, tq):
                        r = phase_idx(npair - 1, j) + 1
                        for it in range(4 * (j - 1), 4 * j):
                            for op in wo_ops(it):
                                work.append((r, True, op))
                    for it in range(4 * (tq - 1), 4 * tq):
                        for op in wo_ops(it, evict_act=True):
                            work.append((nphase + 1, True, op))

                    def drain(limit):
                        while work and work[0][0] <= limit:
                            work.popleft()[2][1]()

                    def pop_pe(budget, limit):
                        # Only WO ops carry an emission-order hazard (they
                        # must not be emitted before the phase whose norm
                        # writes their ot block), so the lookahead cap
                        # req <= current+1 applies to them alone; hazard-
                        # free proj/V ops pop arbitrarily far ahead. Break
                        # (not skip) on a blocked hazard op so FIFO chain
                        # contiguity is preserved.
                        done = 0
                        while work and done < budget:
                            req, hazard, (is_pe, op) = work[0]
                            if hazard and req > limit:
                                break
                            work.popleft()
                            op()
                            if is_pe:
                                done += 1

                    # ---- attention ----------------------------------------
                    def emit_attention_j(
                        pr, j, allow_pops, budget, deferred, last=False
                    ):
                        hA, hB = 2 * pr, 2 * pr + 1
                        qlo = j * 512
                        n_i = 4 * j + 4
                        lim = phase_idx(pr, j) + 1
                        o_a = opsum.tile([128, 512], f32, tag="o")
                        o_b = opsum.tile([128, 512], f32, tag="o")

                        def emit_pv(i, pt, off):
                            nc.tensor.matmul(
                                o_a[0:65, off:512],
                                lhsT=v_sb[:, i, hA, :],
                                rhs=pt[:, off:512],
                                start=(i == 0),
                                stop=(i == n_i - 1),
                            )
                            nc.tensor.matmul(
                                o_b[0:65, off:512],
                                lhsT=v_sb[:, i, hB, :],
                                rhs=pt[:, 512 + off : 1024],
                                start=(i == 0),
                                stop=(i == n_i - 1),
                            )

                        # PV lags TWO i-iterations behind S: every consumer
                        # of pt (exp on ACT, trim on Pool) then has over a
                        # full i-period of slack before the in-order PE
                        # queue reaches the PV that reads it.
                        pending = deque()
                        for i in range(n_i):
                            off = 128 * (i % 4) if i // 4 == j else 0
                            s_ps = spsum.tile([128, 1024], f32, tag="s")
                            # diagonal tiles: stream only q-columns >= off
                            # (below-diagonal columns are all-zero in P and
                            # never touched downstream)
                            nc.tensor.matmul(
                                s_ps[:, off:512],
                                lhsT=kt_sb[0:64, pr, i * 128 : (i + 1) * 128],
                                rhs=qt_sb[0:64, pr, qlo + off : (j + 1) * 512],
                                start=True,
                                stop=True,
                            )
                            nc.tensor.matmul(
                                s_ps[:, 512 + off : 1024],
                                lhsT=kt_sb[64:128, pr, i * 128 : (i + 1) * 128],
                                rhs=qt_sb[64:128, pr, qlo + off : (j + 1) * 512],
                                start=True,
                                stop=True,
                            )
                            pt = ptpool.tile([128, 1024], bf16, tag="pt")
                            if off == 0:
                                nc.scalar.activation(
                                    pt[:, :], s_ps[:, :], Exp, scale=DK ** (-0.5)
                                )
                            else:
                                nc.scalar.activation(
                                    pt[:, off:512],
                                    s_ps[:, off:512],
                                    Exp,
                                    scale=DK ** (-0.5),
                                )
                                nc.scalar.activation(
                                    pt[:, 512 + off : 1024],
                                    s_ps[:, 512 + off : 1024],
                                    Exp,
                                    scale=DK ** (-0.5),
                                )
                            if i // 4 == j:
                                trim_eng = nc.vector if trim_dve else nc.gpsimd
                                trim_eng.tensor_mul(
                                    pt[:, off : off + 128],
                                    pt[:, off : off + 128],
                                    trim_sb,
                                )
                                trim_eng.tensor_mul(
                                    pt[:, 512 + off : 512 + off + 128],
                                    pt[:, 512 + off : 512 + off + 128],
                                    trim_sb,
                                )
                            pending.append((i, pt, off))
                            if len(pending) > lag:
                                emit_pv(*pending.popleft())
                            if allow_pops(i):
                                pop_pe(budget, lim)
                        while pending:
                            emit_pv(*pending.popleft())
                        # normalize both heads for this q-subtile via a
                        # DRAM-bounce partition broadcast of the recip row
                        # (no PE cost; latency hides while the next phase's
                        # i-loop runs). The final chunk instead broadcasts
                        # on the now-idle PE (ones[1,64]^T @ recip into the
                        # already-read o psum) so the tail WO tiles are not
                        # stalled behind a ~4us bounce.
                        for hx, o_ps, po in ((hA, o_a, 0), (hB, o_b, 64)):
                            # fast bf16 eviction releases the o psum slot in
                            # ~0.5us instead of holding it through the
                            # DMA-bounce chain; the final multiply is then
                            # all-SBUF bf16 (DVE packed fast path)
                            og = normp.tile([65, 512], bf16, tag="og")
                            with nc.allow_low_precision(
                                reason="softmax staging bf16"
                            ):
                                nc.vector.tensor_copy(og, o_ps[0:65, 0:512])
                                recip = normp.tile([1, 512], bf16, tag="recip")
                                nc.vector.reciprocal(
                                    recip[0:1, :], og[64:65, :]
                                )
                            if last:
                                nc.tensor.matmul(
                                    o_ps[0:64, 0:512],
                                    lhsT=ones_sb[0:1, 0:64],
                                    rhs=recip[0:1, :],
                                    start=True,
                                    stop=True,
                                )
                                nc.vector.tensor_mul(
                                    ot_sb[
                                        po : po + 64,
                                        pr,
                                        j * 512 : (j + 1) * 512,
                                    ],
                                    og[0:64, :],
                                    o_ps[0:64, 0:512],
                                )
                                continue
                            nc.sync.dma_start(
                                scr[hx : hx + 1, j * 512 : (j + 1) * 512],
                                recip[0:1, :],
                            )
                            bc_sb = normp.tile([64, 512], bf16, tag="bc")
                            scr_bcast = bass.AP(
                                tensor=scr.ap().tensor,
                                offset=hx * t + j * 512,
                                ap=[[0, 64], [1, 512]],
                            )
                            nc.sync.dma_start(bc_sb, scr_bcast)
                            # Pool, not DVE: DVE carries the og/recip and
                            # WO-evict chains in the pair-3 stretches
                            mul_eng = nc.gpsimd if pool_mul else nc.vector
                            mul_eng.tensor_mul(
                                ot_sb[po : po + 64, pr, j * 512 : (j + 1) * 512],
                                og[0:64, :],
                                bc_sb,
                            )

                    pwo = pops_per_i if pops_wo is None else pops_wo
                    for pr in range(npair):
                        for j in range(tq):
                            p = phase_idx(pr, j)
                            drain(p)
                            last = pr == npair - 1 and j == tq - 1
                            if pr == npair - 1:
                                # hold pops while the previous chunk's norm
                                # bounce lands (WO ops are queue-next here)
                                allow = lambda i: i >= wo_delay  # noqa: E731
                                budget = pops_last if last else pwo
                            else:
                                allow = lambda i: True  # noqa: E731
                                budget = pops_per_i
                            emit_attention_j(
                                pr,
                                j,
                                allow,
                                budget,
                                None,
                                last=last,
                            )
                    drain(nphase + 1)
    nc.compile()
    return nc


def _get_nc():
    if "nc" not in _cache:
        _cache["nc"] = _build()
    return _cache["nc"]


def _host_tables(positions):
    """cos/sin RoPE tables laid out for the on-chip [128, T] tiles."""
    pos = np.asarray(positions, np.float32)  # [t]
    inv = 1.0 / THETA ** (
        (2.0 * np.arange(1, DK // 2 + 1, dtype=np.float32) - 2.0) / DK
    )  # [32]
    ang = pos[None, :] * inv[:, None]  # [32, t]
    c32 = np.cos(ang)
    s32 = np.sin(ang)
    rows = np.arange(128)
    dloc = rows % DK
    fidx = dloc // 2
    sign = np.where(dloc % 2 == 0, -1.0, 1.0).astype(np.float32)
    cosT = c32[fidx, :]
    sinT = sign[:, None] * s32[fidx, :]
    return np.ascontiguousarray(cosT), np.ascontiguousarray(sinT)


def _make_in_maps(inputs):
    x = np.asarray(inputs["x"], np.float32)
    token_positions = np.asarray(inputs["token_positions"])
    WQ = np.asarray(inputs["WQ"], np.float32)
    WK = np.asarray(inputs["WK"], np.float32)
    WV = np.asarray(inputs["WV"], np.float32)
    WO = np.asarray(inputs["WO"], np.float32)

    # per-head-group weight shards (shared across batches)
    wsh = {}
    for hg in range(2):
        sl = slice(hg * E, (hg + 1) * E)
        wsh[hg] = (
            np.ascontiguousarray(WQ[sl, :].T).astype(_BF16),
            np.ascontiguousarray(WK[sl, :].T).astype(_BF16),
            np.ascontiguousarray(WV[sl, :].T).astype(_BF16),
            np.ascontiguousarray(WO[:, sl].T).astype(_BF16),
        )
    xts = {b: np.ascontiguousarray(x[b].T).astype(_BF16) for b in range(B)}
    tabs = {}
    for b in range(B):
        key = token_positions[b].tobytes()
        if key not in tabs:
            cosT, sinT = _host_tables(token_positions[b])
            tabs[key] = (cosT.astype(_BF16), sinT.astype(_BF16))

    in_maps = []
    for c in range(NCORES):
        b, hg = c // 2, c % 2
        cosT, sinT = tabs[token_positions[b].tobytes()]
        wq, wk, wv, wo = wsh[hg]
        pkb = np.zeros((_PK_ROWS, _PK_COLS), _BF16)
        pkb[_ROW_X : _ROW_X + D, :] = xts[b]
        pkb[_ROW_W : _ROW_W + D, 0:E] = wq
        pkb[_ROW_W : _ROW_W + D, E : 2 * E] = wk
        pkb[_ROW_W : _ROW_W + D, 2 * E : 3 * E] = wv
        for dh in range(2):
            pkb[_ROW_W + dh * E : _ROW_W + (dh + 1) * E, 3 * E : 4 * E] = wo[
                :, dh * E : (dh + 1) * E
            ]
        pkb[_ROW_TBL : _ROW_TBL + 64, :] = cosT[0:64]
        pkb[_ROW_TBL + 64 : _ROW_TBL + 128, :] = sinT[0:64]
        in_maps.append({"pk": pkb})
    return in_maps


def _get_runner():
    """Build (once) a jitted shard_map over the 8 cores for the bass program."""
    if "runner" in _cache:
        return _cache["runner"]

    import jax
    from jax.sharding import Mesh, PartitionSpec
    from jax.experimental.shard_map import shard_map
    from concourse import bass2jax
    from concourse.bass2jax import _bass_exec_p, partition_id_tensor
    import concourse.mybir as mybir

    bass2jax.install_neuronx_cc_hook()
    nc = _get_nc()

    partition_name = nc.partition_id_tensor.name if nc.partition_id_tensor else None
    in_names, out_names, out_avals = [], [], []
    for alloc in nc.m.functions[0].allocations:
        if not isinstance(alloc, mybir.MemoryLocationSet):
            continue
        name = alloc.memorylocations[0].name
        if alloc.kind == "ExternalInput":
            if name != partition_name:
                in_names.append(name)
        elif alloc.kind == "ExternalOutput":
            out_names.append(name)
            np_dt = mybir.dt.np(alloc.dtype)
            out_avals.append(jax.core.ShapedArray(tuple(alloc.tensor_shape), np_dt))
    all_names = list(in_names)
    if partition_name is not None:
        all_names = all_names + [partition_name]

    # outputs are NOT passed as zero-buffer operands: the kernel writes
    # every element of y, so the custom call's runtime-allocated (uninit)
    # results are fine, and skipping the placeholder saves one I/O buffer
    # binding plus its bytes per call
    def _body(*args):
        operands = list(args)
        if partition_name is not None:
            operands.append(partition_id_tensor())
        return tuple(
            _bass_exec_p.bind(
                *operands,
                out_avals=tuple(out_avals),
                in_names=tuple(all_names),
                out_names=tuple(out_names),
                lowering_input_output_aliases=(),
                sim_require_finite=True,
                sim_require_nnan=True,
                nc=nc,
            )
        )

    devices = jax.devices()[:NCORES]
    mesh = Mesh(np.asarray(devices), ("core",))
    sharded = jax.jit(
        shard_map(
            _body,
            mesh=mesh,
            in_specs=(PartitionSpec("core"),) * len(in_names),
            out_specs=(PartitionSpec("core"),) * len(out_names),
            check_rep=False,
        ),
        keep_unused=True,
    )
    _cache["runner"] = (sharded, in_names, out_names, [])
    return _cache["runner"]


def kernel(x, token_positions, WQ, WK, WV, WO):
    in_maps = _make_in_maps(
        {
            "x": x,
            "token_positions": token_positions,
            "WQ": WQ,
            "WK": WK,
            "WV": WV,
            "WO": WO,
        }
    )
    sharded, in_names, out_names, concat_zeros = _get_runner()
    concat_in = [
        np.concatenate([np.asarray(in_maps[c][nm]) for c in range(NCORES)], axis=0)
        for nm in in_names
    ]
    out_arrs = sharded(*concat_in, *concat_zeros)
    ys = (
        np.asarray(out_arrs[out_names.index("y")])
        .astype(np.float32)
        .reshape(NCORES, T, D)
    )
    out = np.empty((B, T, D), np.float32)
    for b in range(B):
        out[b] = ys[2 * b] + ys[2 * b + 1]
    return out


# revision 48
# speedup vs baseline: 1.0505x; 1.0505x over previous
"""Causal multi-head self-attention with RoPE on 8 NeuronCores.

Sharding (hardcoded): core c -> batch b = c // 2, head-group hg = c % 2.
Each core:
  - projects its batch's x with column-sharded WQ/WK/WV (8 heads = 512 dims),
  - applies RoPE (host-precomputed bf16 cos/sin tables; adjacent-pair
    swap via stream_shuffle),
  - runs causal attention for its 8 heads in transposed layout
    (S^T = [k, q]; softmax denominator comes free from a ones-column
    appended to V; normalization is broadcast via a DRAM bounce),
  - applies the row-sharded WO projection -> bf16 partial [T, D] output.
Host sums the two partials per batch (the "all-reduce after WO").

The schedule is a fine-grained software pipeline built around the
attention i-loop, which is paced by the ACT exp stream (~1.04us/i vs
~0.85us/i of S+PV matmuls on PE). All projection / V / WO matmuls are
split into single-matmul micro-ops on a FIFO work queue tagged with
the attention phase they must precede; each i-iteration of every
attention chunk pops up to pops_per_i PE micro-ops off the queue,
keeping PE busy through the exp-paced stretches. Only WO ops carry a
lookahead cap (req <= phase+1: they must never be emitted before the
norm that writes their ot block); hazard-free proj/V ops pop
arbitrarily far ahead, so their RoPE/evict chains finish well before
their attention consumers. PV matmuls lag `lag` i-iterations behind the
S matmuls so the exp/trim -> PV dependency never head-of-line blocks
the in-order PE queue. PSUM: the S ring (2x2 banks) and the o_a/o_b
accumulators (2x1) are separate pools from the filler chains (2x1),
so a popped filler allocation never waits on an o tile held through
an i-loop. Input DMA is consolidated into ~14 multi-dim copies
(HWDGE descriptor-generation is a fixed cost per copy and paces the
prologue); the final chunk's softmax normalization broadcasts on the
then-idle PE (ones[1,64]^T @ recip) instead of the DRAM bounce so
the tail WO tiles are not stalled behind it.

All inputs are packed into ONE DRAM tensor per core and the output is
not passed as a zero-placeholder operand (the kernel writes every
element, so the runtime-allocated uninit result buffer is fine): the
per-call execution overhead through the axon PJRT path scales with the
number of I/O buffers (~65us each) plus ~15us/MB, so both buffer count
and bytes are minimized. Packed layout (bf16, [2176, 2048]):
  rows    0:1024  xT (x[b].T, [d, t])
  rows 1024:2048  wqT | wkT | wvT | woT-halves (cols 0:512, 512:1024,
                  1024:1536, 1536:2048)
  rows 2048:2176  cos | sin ([64, 2048] each; table partition rows repeat
                  with period 64, so each is DMA'd to both halves)
The triangular trim mask is generated on-chip (memset + affine_select).
"""

import numpy as np
import ml_dtypes

B, T, D, H = 4, 2048, 1024, 16
DK = 64
HLOC = 8          # heads per core
E = HLOC * DK     # 512, local projection width
NCORES = 8
THETA = 10000.0

_BF16 = ml_dtypes.bfloat16

# packed-input row offsets. WO's two column-halves sit in the otherwise
# unused cols 1536:2048 of the W row-block; the triangular trim mask is
# generated on-chip with affine_select, so the buffer stays tight.
_ROW_X = 0
_ROW_W = 1024
_ROW_TBL = 2048
_PK_ROWS = 2176
_PK_COLS = 2048

_cache = {}


def _build(
    t=T,
    hloc=HLOC,
    d=D,
    reps=1,
    pops_per_i=2,
    pops_wo=None,
    pops_last=2,
    wo_delay=4,
    lag=4,
    dma_halves=False,
    evict_split=True,
    pool_mul=False,
    rope_add_pool=False,
    trim_dve=True,
    dma_fine=False,
    dma_hybrid=False,
    vfin_pool=False,
    warmup_mm=0,
):
    from collections import deque
    from contextlib import ExitStack

    import concourse.bacc as bacc
    import concourse.bass as bass  # noqa: F401
    import concourse.mybir as mybir
    import concourse.tile as tile

    f32 = mybir.dt.float32
    bf16 = mybir.dt.bfloat16
    Exp = mybir.ActivationFunctionType.Exp
    Copy = mybir.ActivationFunctionType.Copy

    e = hloc * DK
    npair = hloc // 2       # head-pair tiles in QT/KT/OT
    dsub = d // 128         # contraction subtiles for projections
    tq = t // 512           # 512-wide q chunks
    tk = t // 128           # 128-wide k tiles
    swap_mask = [i ^ 1 for i in range(32)]
    e_v = min(512, e)

    nc = bacc.Bacc(
        "TRN2", target_bir_lowering=False, debug=False, enable_partition_id=False
    )

    pk = nc.declare_dram_parameter("pk", [_PK_ROWS, _PK_COLS], bf16, False).ap()
    y = nc.declare_dram_parameter("y", [t, d], bf16, True).ap()
    scr = nc.dram_tensor("scr", [hloc, t], bf16)  # denom-recip bounce for bcast

    def pk_view(row, col, ap_dims):
        """Multi-dim view into the packed DRAM input: one DMA per logical
        tensor instead of one per 128-row slab (HWDGE cost is per copy)."""
        return bass.AP(
            tensor=pk.tensor, offset=row * _PK_COLS + col, ap=list(ap_dims)
        )

    with tile.TileContext(nc) as tc:
        with ExitStack() as ctx:
            const = ctx.enter_context(tc.tile_pool(name="const", bufs=1))
            ptpool = ctx.enter_context(tc.tile_pool(name="ptp", bufs=8))
            normp = ctx.enter_context(tc.tile_pool(name="normp", bufs=4))
            ysbp = ctx.enter_context(tc.tile_pool(name="ysbp", bufs=3))

            wq_sb = const.tile([128, dsub, e], bf16)
            wk_sb = const.tile([128, dsub, e], bf16)
            wv_sb = const.tile([128, dsub, e], bf16)
            wo_sb = const.tile([128, e // 128, d], bf16)
            trim_sb = const.tile([128, 128], bf16)
            ones_sb = const.tile([1, 64], bf16)
            qt_sb = const.tile([128, npair, t], bf16)
            kt_sb = const.tile([128, npair, t], bf16)
            v_sb = const.tile([128, tk, hloc, DK + 1], bf16)
            ot_sb = const.tile([128, npair, t], bf16)

            # DMA order follows first use in the pipelined schedule:
            # wq -> xt chunk0 -> wk -> wv -> rope tables -> xt rest -> trim
            # -> wo. Q/K/V projections and pair-0 attention start while the
            # later loads are still in flight.
            xt_sb = const.tile([128, dsub, t], bf16)
            cos_sb = const.tile([128, t], bf16)
            sin_sb = const.tile([128, t], bf16)
            tch = 512
            # trim[r, c] = 1.0 if c >= r else 0.0 (upper triangle), built
            # on-chip
            nc.vector.memset(trim_sb, 1.0)
            nc.vector.memset(ones_sb, 1.0)
            nc.gpsimd.affine_select(
                trim_sb,
                trim_sb,
                pattern=[[1, 128]],
                compare_op=mybir.AluOpType.is_ge,
                fill=0.0,
                base=0,
                channel_multiplier=-1,
            )
            # whole-tensor DMAs via multi-dim views: HWDGE descriptor
            # generation is a fixed ~0.6us per COPY, and the issue stream
            # paces the whole prologue, so copy count is minimized (14
            # input copies). Order follows first use: wq+x chunk0 (Q0),
            # tables (Q0's RoPE), wk (K0), wv (V tiles), x rest, wo.
            row = [_PK_COLS, 128]
            dsl = [128 * _PK_COLS, dsub]
            if dma_fine or dma_hybrid:
                # per-slab copies for the prologue gate: the first Q0
                # matmul only needs wq[ds0]+x[ds0] (256KB), not the full
                # 2MB consolidated pair
                for ds_ in range(dsub):
                    r = _ROW_W + ds_ * 128
                    nc.sync.dma_start(wq_sb[:, ds_, :], pk[r : r + 128, 0:e])
                    nc.sync.dma_start(
                        xt_sb[:, ds_, 0:tch],
                        pk[ds_ * 128 : (ds_ + 1) * 128, 0:tch],
                    )
            elif dma_halves:
                hd = dsub // 2
                dsl_h = [128 * _PK_COLS, hd]
                # wq/x chunk0 in interleaved halves so the first projection
                # accumulation starts after ~1MB instead of 2MB
                nc.sync.dma_start(
                    wq_sb[:, 0:hd, :], pk_view(_ROW_W, 0, [row, dsl_h, [1, e]])
                )
                nc.sync.dma_start(
                    xt_sb[:, 0:hd, 0:tch], pk_view(0, 0, [row, dsl_h, [1, tch]])
                )
                nc.sync.dma_start(
                    wq_sb[:, hd:dsub, :],
                    pk_view(_ROW_W + hd * 128, 0, [row, dsl_h, [1, e]]),
                )
                nc.sync.dma_start(
                    xt_sb[:, hd:dsub, 0:tch],
                    pk_view(hd * 128, 0, [row, dsl_h, [1, tch]]),
                )
            else:
                nc.sync.dma_start(
                    wq_sb[:, :, :], pk_view(_ROW_W, 0, [row, dsl, [1, e]])
                )
                nc.sync.dma_start(
                    xt_sb[:, :, 0:tch], pk_view(0, 0, [row, dsl, [1, tch]])
                )
            # tables next: Q0's RoPE finisher (DVE) needs cos/sin as soon
            # as the first projection chunk's matmuls finish
            nc.sync.dma_start(cos_sb[0:64, :], pk[_ROW_TBL : _ROW_TBL + 64, :])
            nc.sync.dma_start(cos_sb[64:128, :], pk[_ROW_TBL : _ROW_TBL + 64, :])
            nc.sync.dma_start(
                sin_sb[0:64, :], pk[_ROW_TBL + 64 : _ROW_TBL + 128, :]
            )
            nc.sync.dma_start(
                sin_sb[64:128, :], pk[_ROW_TBL + 64 : _ROW_TBL + 128, :]
            )
            if dma_fine:
                for ds_ in range(dsub):
                    r = _ROW_W + ds_ * 128
                    nc.sync.dma_start(
                        wk_sb[:, ds_, :], pk[r : r + 128, e : 2 * e]
                    )
                for ds_ in range(dsub):
                    r = _ROW_W + ds_ * 128
                    nc.sync.dma_start(
                        wv_sb[:, ds_, :], pk[r : r + 128, 2 * e : 3 * e]
                    )
                nc.vector.memset(v_sb[:, :, :, DK : DK + 1], 1.0)
                for t0 in range(tch, t, tch):
                    for ds_ in range(dsub):
                        nc.sync.dma_start(
                            xt_sb[:, ds_, t0 : t0 + tch],
                            pk[ds_ * 128 : (ds_ + 1) * 128, t0 : t0 + tch],
                        )
                for dp in range(e // 128):
                    for dh in range(2):
                        r = _ROW_W + dh * 512 + dp * 128
                        nc.sync.dma_start(
                            wo_sb[:, dp, dh * 512 : (dh + 1) * 512],
                            pk[r : r + 128, 3 * e : 4 * e],
                        )
            else:
                nc.sync.dma_start(
                    wk_sb[:, :, :], pk_view(_ROW_W, e, [row, dsl, [1, e]])
                )
                nc.sync.dma_start(
                    wv_sb[:, :, :], pk_view(_ROW_W, 2 * e, [row, dsl, [1, e]])
                )
                nc.vector.memset(v_sb[:, :, :, DK : DK + 1], 1.0)
                nc.sync.dma_start(
                    xt_sb[:, :, tch : 2 * tch],
                    pk_view(0, tch, [row, dsl, [1, tch]]),
                )
                nc.sync.dma_start(
                    xt_sb[:, :, 2 * tch : t],
                    pk_view(0, 2 * tch, [row, dsl, [1, t - 2 * tch]]),
                )
                for dh in range(2):
                    nc.sync.dma_start(
                        wo_sb[:, :, dh * 512 : (dh + 1) * 512],
                        pk_view(
                            _ROW_W + dh * 512,
                            3 * e,
                            [row, [128 * _PK_COLS, e // 128], [1, 512]],
                        ),
                    )

            for _rep in range(reps):
                with ExitStack() as c1:
                    rope = c1.enter_context(tc.tile_pool(name="rope", bufs=3))
                    # PSUM budget (8 banks): "s" 2x[128,1024] = 4 banks for
                    # attention S^T psum; "o" 2x[128,512] = 2 banks for the
                    # attention o accumulators; "f" 2x[128,512] = 2 banks
                    # for the filler chains (projection / V / WO psum).
                    # Separate o and filler rings keep a popped filler's
                    # allocation from ever waiting on an o-tile held through
                    # a whole i-loop; the FIFO keeps chains contiguous so at
                    # most 2 filler tiles are ever in flight.
                    spsum = c1.enter_context(
                        tc.tile_pool(name="spsum", bufs=2, space="PSUM")
                    )
                    opsum = c1.enter_context(
                        tc.tile_pool(name="opsum", bufs=2, space="PSUM")
                    )
                    fpsum = c1.enter_context(
                        tc.tile_pool(name="fpsum", bufs=2, space="PSUM")
                    )

                    ph = min(512, t)           # projection t-chunk (1 PSUM bank)
                    nh = t // ph               # chunks per e-tile

                    if _rep == 0 and warmup_mm:
                        # p-state warm-up: PE idles ~6us at the start
                        # waiting for the first wq/x DMAs, and the DVFS
                        # ramp then slows the first real matmuls (full
                        # clock only after ~3us of continuous busy). Run
                        # dummy matmuls on a zeroed tile through the
                        # otherwise-dead ramp window so the real stream
                        # starts at speed.
                        wz = rope.tile([128, 512], bf16, tag="sw2", name="wz")
                        nc.vector.memset(wz, 0.0)
                        wu_ps = fpsum.tile([128, 512], f32, tag="f", name="wu_ps")
                        for wi in range(warmup_mm):
                            nc.tensor.matmul(
                                wu_ps[:, :],
                                lhsT=wz[:, 0:128],
                                rhs=wz[:, :],
                                start=(wi == 0),
                                stop=(wi == warmup_mm - 1),
                            )
                        nc.vector.tensor_copy(wz, wu_ps)

                    # ---- micro-op generators -------------------------------
                    # Each returns a list of (is_pe, closure). Ops of one
                    # chain stay contiguous in the FIFO, so at most one
                    # extra PSUM po tile is in flight beyond o_a/o_b.

                    def proj_ops(pr, which, ch):
                        """Q or K projection chunk + RoPE, split into dsub
                        single matmuls plus one finisher op (DVE shuffle +
                        cos-mul, Pool sin-mul, DVE add)."""
                        wsb, dst = (
                            (wq_sb, qt_sb) if which == 0 else (wk_sb, kt_sb)
                        )
                        c0 = ch * ph
                        st = {}

                        def mk_mm(ds_):
                            def f():
                                if ds_ == 0:
                                    st["ps"] = fpsum.tile([128, ph], f32, tag="f", name="q_ps")
                                nc.tensor.matmul(
                                    st["ps"][:, :],
                                    lhsT=wsb[:, ds_, pr * 128 : (pr + 1) * 128],
                                    rhs=xt_sb[:, ds_, c0 : c0 + ph],
                                    start=(ds_ == 0),
                                    stop=(ds_ == dsub - 1),
                                )

                            return f

                        def fin():
                            q_ps = st["ps"]
                            sw = rope.tile([128, ph], f32, tag="sw")
                            nc.vector.stream_shuffle(sw, q_ps, mask=swap_mask)
                            nc.vector.tensor_mul(
                                dst[:, pr, c0 : c0 + ph],
                                q_ps,
                                cos_sb[:, c0 : c0 + ph],
                            )
                            # Pool does the sin product AND the final add:
                            # DVE per chunk (shuffle + cos-mul ~1.6us) then
                            # stays under PE's 1.7us/chunk, so filler bursts
                            # never stall the PE on the fpsum ring.
                            sw2 = rope.tile([128, ph], bf16, tag="sw2")
                            nc.gpsimd.tensor_mul(
                                sw2, sw, sin_sb[:, c0 : c0 + ph]
                            )
                            add_eng = nc.gpsimd if rope_add_pool else nc.vector
                            add_eng.tensor_add(
                                dst[:, pr, c0 : c0 + ph],
                                dst[:, pr, c0 : c0 + ph],
                                sw2,
                            )

                        return [(True, mk_mm(ds_)) for ds_ in range(dsub)] + [
                            (False, fin)
                        ]

                    def v_ops(it):
                        """V projection tile: dsub matmuls into a po tile +
                        ACT psum eviction (Copy shares the exp table, so no
                        act-table reloads)."""
                        st = {}

                        def mk_mm(ds_):
                            def f():
                                if ds_ == 0:
                                    st["ps"] = fpsum.tile(
                                        [128, e_v], f32, tag="f", name="v_ps"
                                    )
                                nc.tensor.matmul(
                                    st["ps"][:, :],
                                    lhsT=xt_sb[:, ds_, it * 128 : (it + 1) * 128],
                                    rhs=wv_sb[:, ds_, :e_v],
                                    start=(ds_ == 0),
                                    stop=(ds_ == dsub - 1),
                                )

                            return f

                        def fin():
                            if vfin_pool:
                                # keep the eviction off the ACT exp stream
                                # (pair-0 ACT runs at ~the PE pace already)
                                nc.gpsimd.tensor_copy(
                                    v_sb[:, it, :, 0:DK],
                                    st["ps"][:, :].rearrange(
                                        "p (h k) -> p h k", h=hloc
                                    ),
                                )
                            else:
                                nc.scalar.activation(
                                    v_sb[:, it, :, 0:DK],
                                    st["ps"][:, :].rearrange(
                                        "p (h k) -> p h k", h=hloc
                                    ),
                                    Copy,
                                )

                        return [(True, mk_mm(ds_)) for ds_ in range(dsub)] + [
                            (False, fin)
                        ]

                    def wo_ops(it, evict_act=False):
                        """WO tile for output rows it*128..: two 512-wide
                        column halves, each e//128 accumulating matmuls plus
                        an eviction, then the output DMA."""
                        st = {}
                        ops = []

                        def mk_mm(ec, dp):
                            def f():
                                if ec == 0 and dp == 0:
                                    st["ysb"] = ysbp.tile([128, d], bf16, tag="ysb", name="y_sb")
                                if dp == 0:
                                    st["yps"] = fpsum.tile(
                                        [128, 512], f32, tag="f", name="y_ps"
                                    )
                                nc.tensor.matmul(
                                    st["yps"][:, :],
                                    lhsT=ot_sb[:, dp, it * 128 : (it + 1) * 128],
                                    rhs=wo_sb[:, dp, ec * 512 : (ec + 1) * 512],
                                    start=(dp == 0),
                                    stop=(dp == e // 128 - 1),
                                )

                            return f

                        def mk_fin(ec):
                            def f():
                                if evict_act or (evict_split and ec == 0):
                                    # ec0 on ACT (slack beside the exp
                                    # stream), ec1 on DVE: neither engine
                                    # becomes the pair-3 pacer
                                    nc.scalar.activation(
                                        st["ysb"][:, ec * 512 : (ec + 1) * 512],
                                        st["yps"][:, :],
                                        Copy,
                                    )
                                else:
                                    nc.vector.tensor_copy(
                                        st["ysb"][:, ec * 512 : (ec + 1) * 512],
                                        st["yps"][:, :],
                                    )
                                # per-half DMA starts the output drain while
                                # the second half still computes
                                nc.sync.dma_start(
                                    y[
                                        it * 128 : (it + 1) * 128,
                                        ec * 512 : (ec + 1) * 512,
                                    ],
                                    st["ysb"][:, ec * 512 : (ec + 1) * 512],
                                )

                            return f

                        for ec in range(d // 512):
                            for dp in range(e // 128):
                                ops.append((True, mk_mm(ec, dp)))
                            ops.append((False, mk_fin(ec)))
                        return ops

                    # ---- work queue ---------------------------------------
                    # req = phase index (pr*tq + j) the ops must precede.
                    # Queue order is by req, so FIFO pops respect both PSUM
                    # ring discipline and write-before-read emission order.
                    work = deque()
                    nphase = npair * tq

                    def phase_idx(pr, j):
                        return pr * tq + j

                    for j in range(tq):
                        r = phase_idx(0, j)
                        for op in proj_ops(0, 0, j):
                            work.append((r, False, op))
                        for op in proj_ops(0, 1, j):
                            work.append((r, False, op))
                        for it in range(4 * j, 4 * j + 4):
                            for op in v_ops(it):
                                work.append((r, False, op))
                    for pr in range(1, npair):
                        for ch in range(nh):
                            r = phase_idx(pr, ch)
                            for op in proj_ops(pr, 0, ch):
                                work.append((r, False, op))
                            for op in proj_ops(pr, 1, ch):
                                work.append((r, False, op))
                    # WO for q-chunk j-1 pops during phase (3, j); the
                    # first wo_delay i-iterations don't pop, giving the
                    # norm DRAM-bounce of chunk j-1 time to land.
                    for j in range(1, tq):
                        r = phase_idx(npair - 1, j) + 1
                        for it in range(4 * (j - 1), 4 * j):
                            for op in wo_ops(it):
                                work.append((r, True, op))
                    for it in range(4 * (tq - 1), 4 * tq):
                        for op in wo_ops(it, evict_act=True):
                            work.append((nphase + 1, True, op))

                    def drain(limit):
                        while work and work[0][0] <= limit:
                            work.popleft()[2][1]()

                    def pop_pe(budget, limit):
                        # Only WO ops carry an emission-order hazard (they
                        # must not be emitted before the phase whose norm
                        # writes their ot block), so the lookahead cap
                        # req <= current+1 applies to them alone; hazard-
                        # free proj/V ops pop arbitrarily far ahead. Break
                        # (not skip) on a blocked hazard op so FIFO chain
                        # contiguity is preserved.
                        done = 0
                        while work and done < budget:
                            req, hazard, (is_pe, op) = work[0]
                            if hazard and req > limit:
                                break
                            work.popleft()
                            op()
                            if is_pe:
                                done += 1

                    # ---- attention ----------------------------------------
                    def emit_attention_j(
                        pr, j, allow_pops, budget, deferred, last=False
                    ):
                        hA, hB = 2 * pr, 2 * pr + 1
                        qlo = j * 512
                        n_i = 4 * j + 4
                        lim = phase_idx(pr, j) + 1
                        o_a = opsum.tile([128, 512], f32, tag="o")
                        o_b = opsum.tile([128, 512], f32, tag="o")

                        def emit_pv(i, pt, off):
                            nc.tensor.matmul(
                                o_a[0:65, off:512],
                                lhsT=v_sb[:, i, hA, :],
                                rhs=pt[:, off:512],
                                start=(i == 0),
                                stop=(i == n_i - 1),
                            )
                            nc.tensor.matmul(
                                o_b[0:65, off:512],
                                lhsT=v_sb[:, i, hB, :],
                                rhs=pt[:, 512 + off : 1024],
                                start=(i == 0),
                                stop=(i == n_i - 1),
                            )

                        # PV lags TWO i-iterations behind S: every consumer
                        # of pt (exp on ACT, trim on Pool) then has over a
                        # full i-period of slack before the in-order PE
                        # queue reaches the PV that reads it.
                        pending = deque()
                        for i in range(n_i):
                            off = 128 * (i % 4) if i // 4 == j else 0
                            s_ps = spsum.tile([128, 1024], f32, tag="s")
                            # diagonal tiles: stream only q-columns >= off
                            # (below-diagonal columns are all-zero in P and
                            # never touched downstream)
                            nc.tensor.matmul(
                                s_ps[:, off:512],
                                lhsT=kt_sb[0:64, pr, i * 128 : (i + 1) * 128],
                                rhs=qt_sb[0:64, pr, qlo + off : (j + 1) * 512],
                                start=True,
                                stop=True,
                            )
                            nc.tensor.matmul(
                                s_ps[:, 512 + off : 1024],
                                lhsT=kt_sb[64:128, pr, i * 128 : (i + 1) * 128],
                                rhs=qt_sb[64:128, pr, qlo + off : (j + 1) * 512],
                                start=True,
                                stop=True,
                            )
                            pt = ptpool.tile([128, 1024], bf16, tag="pt")
                            if off == 0:
                                nc.scalar.activation(
                                    pt[:, :], s_ps[:, :], Exp, scale=DK ** (-0.5)
                                )
                            else:
                                nc.scalar.activation(
                                    pt[:, off:512],
                                    s_ps[:, off:512],
                                    Exp,
                                    scale=DK ** (-0.5),
                                )
                                nc.scalar.activation(
                                    pt[:, 512 + off : 1024],
                                    s_ps[:, 512 + off : 1024],
                                    Exp,
                                    scale=DK ** (-0.5),
                                )
                            if i // 4 == j:
                                trim_eng = nc.vector if trim_dve else nc.gpsimd
                                trim_eng.tensor_mul(
                                    pt[:, off : off + 128],
                                    pt[:, off : off + 128],
                                    trim_sb,
                                )
                                trim_eng.tensor_mul(
                                    pt[:, 512 + off : 512 + off + 128],
                                    pt[:, 512 + off : 512 + off + 128],
                                    trim_sb,
                                )
                            pending.append((i, pt, off))
                            if len(pending) > lag:
                                emit_pv(*pending.popleft())
                            if allow_pops(i):
                                pop_pe(budget, lim)
                        while pending:
                            emit_pv(*pending.popleft())
                        # normalize both heads for this q-subtile via a
                        # DRAM-bounce partition broadcast of the recip row
                        # (no PE cost; latency hides while the next phase's
                        # i-loop runs). The final chunk instead broadcasts
                        # on the now-idle PE (ones[1,64]^T @ recip into the
                        # already-read o psum) so the tail WO tiles are not
                        # stalled behind a ~4us bounce.
                        for hx, o_ps, po in ((hA, o_a, 0), (hB, o_b, 64)):
                            # fast bf16 eviction releases the o psum slot in
                            # ~0.5us instead of holding it through the
                            # DMA-bounce chain; the final multiply is then
                            # all-SBUF bf16 (DVE packed fast path)
                            og = normp.tile([65, 512], bf16, tag="og")
                            with nc.allow_low_precision(
                                reason="softmax staging bf16"
                            ):
                                nc.vector.tensor_copy(og, o_ps[0:65, 0:512])
                                recip = normp.tile([1, 512], bf16, tag="recip")
                                nc.vector.reciprocal(
                                    recip[0:1, :], og[64:65, :]
                                )
                            if last:
                                nc.tensor.matmul(
                                    o_ps[0:64, 0:512],
                                    lhsT=ones_sb[0:1, 0:64],
                                    rhs=recip[0:1, :],
                                    start=True,
                                    stop=True,
                                )
                                nc.vector.tensor_mul(
                                    ot_sb[
                                        po : po + 64,
                                        pr,
                                        j * 512 : (j + 1) * 512,
                                    ],
                                    og[0:64, :],
                                    o_ps[0:64, 0:512],
                                )
                                continue
                            nc.sync.dma_start(
                                scr[hx : hx + 1, j * 512 : (j + 1) * 512],
                                recip[0:1, :],
                            )
                            bc_sb = normp.tile([64, 512], bf16, tag="bc")
                            scr_bcast = bass.AP(
                                tensor=scr.ap().tensor,
                                offset=hx * t + j * 512,
                                ap=[[0, 64], [1, 512]],
                            )
                            nc.sync.dma_start(bc_sb, scr_bcast)
                            # Pool, not DVE: DVE carries the og/recip and
                            # WO-evict chains in the pair-3 stretches
                            mul_eng = nc.gpsimd if pool_mul else nc.vector
                            mul_eng.tensor_mul(
                                ot_sb[po : po + 64, pr, j * 512 : (j + 1) * 512],
                                og[0:64, :],
                                bc_sb,
                            )

                    pwo = pops_per_i if pops_wo is None else pops_wo
                    for pr in range(npair):
                        for j in range(tq):
                            p = phase_idx(pr, j)
                            drain(p)
                            last = pr == npair - 1 and j == tq - 1
                            if pr == npair - 1:
                                # hold pops while the previous chunk's norm
                                # bounce lands (WO ops are queue-next here)
                                allow = lambda i: i >= wo_delay  # noqa: E731
                                budget = pops_last if last else pwo
                            else:
                                allow = lambda i: True  # noqa: E731
                                budget = pops_per_i
                            emit_attention_j(
                                pr,
                                j,
                                allow,
                                budget,
                                None,
                                last=last,
                            )
                    drain(nphase + 1)
    nc.compile()
    return nc


def _get_nc():
    if "nc" not in _cache:
        _cache["nc"] = _build()
    return _cache["nc"]


def _host_tables(positions):
    """cos/sin RoPE tables laid out for the on-chip [128, T] tiles."""
    pos = np.asarray(positions, np.float32)  # [t]
    inv = 1.0 / THETA ** (
        (2.0 * np.arange(1, DK // 2 + 1, dtype=np.float32) - 2.0) / DK
    )  # [32]
    ang = pos[None, :] * inv[:, None]  # [32, t]
    c32 = np.cos(ang)
    s32 = np.sin(ang)
    rows = np.arange(128)
    dloc = rows % DK
    fidx = dloc // 2
    sign = np.where(dloc % 2 == 0, -1.0, 1.0).astype(np.float32)
    cosT = c32[fidx, :]
    sinT = sign[:, None] * s32[fidx, :]
    return np.ascontiguousarray(cosT), np.ascontiguousarray(sinT)


def _make_in_maps(inputs):
    x = np.asarray(inputs["x"], np.float32)
    token_positions = np.asarray(inputs["token_positions"])
    WQ = np.asarray(inputs["WQ"], np.float32)
    WK = np.asarray(inputs["WK"], np.float32)
    WV = np.asarray(inputs["WV"], np.float32)
    WO = np.asarray(inputs["WO"], np.float32)

    # per-head-group weight shards (shared across batches)
    wsh = {}
    for hg in range(2):
        sl = slice(hg * E, (hg + 1) * E)
        wsh[hg] = (
            np.ascontiguousarray(WQ[sl, :].T).astype(_BF16),
            np.ascontiguousarray(WK[sl, :].T).astype(_BF16),
            np.ascontiguousarray(WV[sl, :].T).astype(_BF16),
            np.ascontiguousarray(WO[:, sl].T).astype(_BF16),
        )
    xts = {b: np.ascontiguousarray(x[b].T).astype(_BF16) for b in range(B)}
    tabs = {}
    for b in range(B):
        key = token_positions[b].tobytes()
        if key not in tabs:
            cosT, sinT = _host_tables(token_positions[b])
            tabs[key] = (cosT.astype(_BF16), sinT.astype(_BF16))

    in_maps = []
    for c in range(NCORES):
        b, hg = c // 2, c % 2
        cosT, sinT = tabs[token_positions[b].tobytes()]
        wq, wk, wv, wo = wsh[hg]
        pkb = np.zeros((_PK_ROWS, _PK_COLS), _BF16)
        pkb[_ROW_X : _ROW_X + D, :] = xts[b]
        pkb[_ROW_W : _ROW_W + D, 0:E] = wq
        pkb[_ROW_W : _ROW_W + D, E : 2 * E] = wk
        pkb[_ROW_W : _ROW_W + D, 2 * E : 3 * E] = wv
        for dh in range(2):
            pkb[_ROW_W + dh * E : _ROW_W + (dh + 1) * E, 3 * E : 4 * E] = wo[
                :, dh * E : (dh + 1) * E
            ]
        pkb[_ROW_TBL : _ROW_TBL + 64, :] = cosT[0:64]
        pkb[_ROW_TBL + 64 : _ROW_TBL + 128, :] = sinT[0:64]
        in_maps.append({"pk": pkb})
    return in_maps


def _get_runner():
    """Build (once) a jitted shard_map over the 8 cores for the bass program."""
    if "runner" in _cache:
        return _cache["runner"]

    import jax
    from jax.sharding import Mesh, PartitionSpec
    from jax.experimental.shard_map import shard_map
    from concourse import bass2jax
    from concourse.bass2jax import _bass_exec_p, partition_id_tensor
    import concourse.mybir as mybir

    bass2jax.install_neuronx_cc_hook()
    nc = _get_nc()

    partition_name = nc.partition_id_tensor.name if nc.partition_id_tensor else None
    in_names, out_names, out_avals = [], [], []
    for alloc in nc.m.functions[0].allocations:
        if not isinstance(alloc, mybir.MemoryLocationSet):
            continue
        name = alloc.memorylocations[0].name
        if alloc.kind == "ExternalInput":
            if name != partition_name:
                in_names.append(name)
        elif alloc.kind == "ExternalOutput":
            out_names.append(name)
            np_dt = mybir.dt.np(alloc.dtype)
            out_avals.append(jax.core.ShapedArray(tuple(alloc.tensor_shape), np_dt))
    all_names = list(in_names)
    if partition_name is not None:
        all_names = all_names + [partition_name]

    # outputs are NOT passed as zero-buffer operands: the kernel writes
    # every element of y, so the custom call's runtime-allocated (uninit)
    # results are fine, and skipping the placeholder saves one I/O buffer
    # binding plus its bytes per call
    def _body(*args):
        operands = list(args)
        if partition_name is not None:
            operands.append(partition_id_tensor())
        return tuple(
            _bass_exec_p.bind(
                *operands,
                out_avals=tuple(out_avals),
                in_names=tuple(all_names),
                out_names=tuple(out_names),
                lowering_input_output_aliases=(),
                sim_require_finite=True,
                sim_require_nnan=True,
                nc=nc,
            )
        )

    devices = jax.devices()[:NCORES]
    mesh = Mesh(np.asarray(devices), ("core",))
    sharded = jax.jit(
        shard_map(
            _body,
            mesh=mesh,
            in_specs=(PartitionSpec("core"),) * len(in_names),
            out_specs=(PartitionSpec("core"),) * len(out_names),
            check_rep=False,
        ),
        keep_unused=True,
    )
    _cache["runner"] = (sharded, in_names, out_names, [])
    return _cache["runner"]


def kernel(x, token_positions, WQ, WK, WV, WO):
    in_maps = _make_in_maps(
        {
            "x": x,
            "token_positions": token_positions,
            "WQ": WQ,
            "WK": WK,
            "WV": WV,
            "WO": WO,
        }
    )
    sharded, in_names, out_names, concat_zeros = _get_runner()
    concat_in = [
        np.concatenate([np.asarray(in_maps[c][nm]) for c in range(NCORES)], axis=0)
        for nm in in_names
    ]
    out_arrs = sharded(*concat_in, *concat_zeros)
    ys = (
        np.asarray(out_arrs[out_names.index("y")])
        .astype(np.float32)
        .reshape(NCORES, T, D)
    )
    out = np.empty((B, T, D), np.float32)
    for b in range(B):
        out[b] = ys[2 * b] + ys[2 * b + 1]
    return out
